# revision 24
# baseline (speedup 1.0000x reference)
"""Trainium2 Bass kernel for nn_AttentionLayer (B=8, S=2048, D=256, U=128).

Data-parallel over the batch dim: one batch element per NeuronCore, weights
replicated. Per-core flash-attention-style layer in a transpose-free layout.

Sequence relabeling: row s of X lives at (partition p, tile t) with
s = p*NT + t, so DMAs move multi-KB contiguous runs per partition.
Attention is permutation-invariant over sequence position as long as loads,
V/K indexing, residual, and stores use the same relabeling (they do).

Layout/engine plan (v2):
  - X loaded as fp16 via GpSimd casting DMAs (f32->f16 in flight), 4 chunks.
  - X^T built by PE matmuls against an fp16 identity (regular matmul, not
    transpose-mode: streams at ~N cycles and stays in the warm-clock path).
  - Q^T/K^T with W stationary (N=512 moving), V natural with X^T stationary.
  - Scores S^T = K_tile^T^T . Q^T into fp32 PSUM [128,1024] (2 banks),
    exp on ScalarE -> fp16 SBUF (the kernel's pace-setter: 32 x ~1.07us).
  - Row-sum accumulator racc (fp16) += e on DVE: 16-bit packed operands hit
    the 2x_1p DVE mode, one [128,1024] add per key tile.
  - O^T += V_tile^T . E accumulated in fp32 PSUM across key tiles.
  - Finish per 1024-query pair: O^T -> SBUF fp16, row sums via ones-matmul,
    transpose to [q,1] via K=1 matmuls, reciprocal, per-tile projection
    matmul + fused (proj*recip + residual) on DVE, chunked stores.
"""

import sys

if "/opt/trn_rl_repo" not in sys.path:
    sys.path.insert(0, "/opt/trn_rl_repo")

from contextlib import ExitStack

import numpy as np

import concourse.bass as bass
import concourse.tile as tile
from concourse import bacc, mybir
from concourse.bass_utils import run_bass_kernel_spmd
from concourse.masks import make_identity

B, S, D, U, P = 8, 2048, 256, 128, 128
NT = S // P            # 16 sequence tiles of 128
QC = 512               # query chunk (one PSUM bank of fp32)
NPAIR = 2              # two 1024-query pairs
SCALE = 1.0 / float(np.sqrt(U))
F32 = mybir.dt.float32
F16 = mybir.dt.float16
EXP = mybir.ActivationFunctionType.Exp
N_WARMUP = 36          # PE activity while DMAs fly, for HAM warm-up
E_BUFS = 8             # exp output lookahead buffers
GPS_X_DMA = True       # X via gpsimd casting DMA (else SP f32 + DVE cast)
GPS_XRES = True        # residual+bias add on GpSimd (else DVE)
GPS_STT = False        # tail scale+residual split onto GpSimd


def build_bass():
    nc = bacc.Bacc("TRN2", target_bir_lowering=False, debug=False)

    x = nc.dram_tensor("inputs", [S, D], F32, kind="ExternalInput").ap()
    wq_d = nc.dram_tensor("W_q", [D, U], F32, kind="ExternalInput").ap()
    wk_d = nc.dram_tensor("W_k", [D, U], F32, kind="ExternalInput").ap()
    wv_d = nc.dram_tensor("W_v", [D, U], F32, kind="ExternalInput").ap()
    wo_d = nc.dram_tensor("W_o", [U, D], F32, kind="ExternalInput").ap()
    bo_d = nc.dram_tensor("b_o", [D], F32, kind="ExternalInput").ap()
    out_d = nc.dram_tensor("out", [S, D], F32, kind="ExternalOutput").ap()

    # s = p*NT + t: contiguous per-partition runs for every DMA
    x_tiled = x.rearrange("(p t) d -> p t d", t=NT)
    out_tiled = out_d.rearrange("(p t) d -> p t d", t=NT)

    with tile.TileContext(nc) as tc, ExitStack() as ctx:
        consts = ctx.enter_context(tc.tile_pool(name="consts", bufs=1))
        sb = ctx.enter_context(tc.tile_pool(name="sb", bufs=1))
        work = ctx.enter_context(tc.tile_pool(name="work", bufs=E_BUFS))
        outp = ctx.enter_context(tc.tile_pool(name="outp", bufs=2))
        # PSUM budget (8 banks): sc 2x[128,1024]f32 = 4, ot 1x[128,1024]f32
        # = 2, misc 2x[128,512]f32 = 2.
        ps_sc = ctx.enter_context(tc.tile_pool(name="ps_sc", bufs=2, space="PSUM"))
        ps_ot = ctx.enter_context(tc.tile_pool(name="ps_ot", bufs=1, space="PSUM"))
        ps_misc = ctx.enter_context(tc.tile_pool(name="ps_misc", bufs=2, space="PSUM"))

        # ---- tiny constants (DVE memsets, cheap) ----
        zbias = consts.tile([P, 1], F32)
        nc.vector.memset(zbias, 0.0)
        ones_h = consts.tile([P, 1], F16)
        nc.vector.memset(ones_h, 1.0)
        wu_sb = consts.tile([P, P], F16)
        nc.vector.memset(wu_sb, 0.0)

        # ---- loads: everything through GpSimd casting DMAs (f32->f16 in
        # flight, software DGE keeps the HWDGE path free).  Order matters:
        # identity first (gates the transposes), then x chunks 0/1, then
        # Wq/Wk/Wv (needed by group 0's projections), then the rest.
        x16 = sb.tile([P, NT, D], F16)
        # chunk 0 is the critical path: fast HWDGE f32 load + DVE cast beats
        # the software-DGE casting DMA's ~2us extra latency
        x_nat0 = consts.tile([P, 4, D], F32)
        nc.sync.dma_start(out=x_nat0[:], in_=x_tiled[:, 0:4, :])

        ident_h = consts.tile([P, P], F16)
        make_identity(nc, ident_h)

        wq16 = consts.tile([P, 2, U], F16)
        wk16 = consts.tile([P, 2, U], F16)
        wv16 = consts.tile([P, 2, U], F16)
        wo16 = consts.tile([P, D], F16)
        bo16 = consts.tile([P, D], F16)
        bo_bcast = bass.AP(tensor=bo_d.tensor, offset=bo_d.offset,
                           ap=[[0, P]] + list(bo_d.ap))

        nc.gpsimd.dma_start(out=wq16[:], in_=wq_d.rearrange("(c p) u -> p c u", p=P))
        nc.gpsimd.dma_start(out=x16[:, 4:8, :], in_=x_tiled[:, 4:8, :])
        nc.gpsimd.dma_start(out=wk16[:], in_=wk_d.rearrange("(c p) u -> p c u", p=P))
        nc.gpsimd.dma_start(out=wv16[:], in_=wv_d.rearrange("(c p) u -> p c u", p=P))
        for g in (2, 3):
            sl = slice(4 * g, 4 * (g + 1))
            nc.gpsimd.dma_start(out=x16[:, sl, :], in_=x_tiled[:, sl, :])
        nc.gpsimd.dma_start(out=wo16[:], in_=wo_d)
        nc.gpsimd.dma_start(out=bo16[:], in_=bo_bcast)
        # split cast across DVE+ScalarE so group 0 unblocks sooner
        nc.vector.tensor_copy(x16[:, 0:2, :], x_nat0[:, 0:2, :])
        nc.scalar.copy(x16[:, 2:4, :], x_nat0[:, 2:4, :])

        # ---- PE warmup while DMAs are in flight ----
        wu_ps = ps_misc.tile([P, P], F32, tag="misc")
        for _ in range(N_WARMUP):
            nc.tensor.matmul(wu_ps[:], wu_sb[:], wu_sb[:], start=True, stop=True)

        # ---- residual (x + b_o) in fp16, off the critical path ----
        xres16 = sb.tile([P, NT, D], F16)
        bo_rep = bass.AP(tensor=bo16.tensor, offset=bo16.offset,
                         ap=[list(bo16.ap[0]), [0, 2]] + list(bo16.ap[1:]))

        def emit_xres(half):
            eng = nc.gpsimd if GPS_XRES else nc.vector
            for i in range(4):
                t0 = half * 8 + 2 * i
                eng.tensor_add(xres16[:, t0:t0 + 2, :],
                               x16[:, t0:t0 + 2, :], bo_rep)

        # ---- per-group: transposes + QKV ----
        xt = sb.tile([P, 2, NT, P], F16)   # X^T chunks: [d_in_chunk, c, t, s]
        qt = sb.tile([P, S], F16)          # Q^T [u, q-col]
        kt16 = sb.tile([P, S], F16)        # K^T [u, k-col]
        v16 = sb.tile([P, NT, U], F16)     # V natural [s_in_tile, t, u]

        def tx_chunk(g, c, copy_eng):
            # 4 transposes via plain matmul against identity into one PSUM
            # bank, then one cast-copy out.
            tsl = slice(4 * g, 4 * (g + 1))
            xtg = ps_misc.tile([P, 4, P], F32, tag="misc")
            for dt in range(4):
                t = 4 * g + dt
                nc.tensor.matmul(xtg[:, dt, :],
                                 x16[:, t, c * P:(c + 1) * P],
                                 ident_h[:], start=True, stop=True)
            copy_eng(xt[:, c, tsl, :], xtg[:])

        def tx_pair(g):
            # head variant: both d-chunks of a group into one (still unused)
            # scores tile, so the misc pool doesn't gate the Q/K matmuls
            tsl = slice(4 * g, 4 * (g + 1))
            xtg = ps_sc.tile([P, 2, 4, P], F32, tag="sc")
            for c in range(2):
                for dt in range(4):
                    t = 4 * g + dt
                    nc.tensor.matmul(xtg[:, c, dt, :],
                                     x16[:, t, c * P:(c + 1) * P],
                                     ident_h[:], start=True, stop=True)
            nc.scalar.copy(xt[:, 0, tsl, :], xtg[:, 0])
            nc.vector.tensor_copy(xt[:, 1, tsl, :], xtg[:, 1])

        def qk_proj(g, w16, dst, copy_eng):
            csl = slice(g * QC, (g + 1) * QC)
            xt2d = xt.rearrange("p c t s -> p c (t s)")
            ps = ps_misc.tile([P, QC], F32, tag="misc")
            nc.tensor.matmul(ps[:], w16[:, 0, :], xt2d[:, 0, csl],
                             start=True, stop=False)
            nc.tensor.matmul(ps[:], w16[:, 1, :], xt2d[:, 1, csl],
                             start=False, stop=True)
            copy_eng(dst[:, csl], ps[:])

        def qkv_qk(g, copy_eng, xt_eng=None):
            tx_chunk(g, 0, copy_eng)
            tx_chunk(g, 1, xt_eng or copy_eng)
            qk_proj(g, wq16, qt, nc.vector.tensor_copy)
            qk_proj(g, wk16, kt16, copy_eng)

        def qkv_v(g, copy_eng):
            tsl = slice(4 * g, 4 * (g + 1))
            vg = ps_misc.tile([P, 4, U], F32, tag="misc")
            for dt in range(4):
                t = 4 * g + dt
                nc.tensor.matmul(vg[:, dt, :], xt[:, 0, t, :], wv16[:, 0, :],
                                 start=True, stop=False)
                nc.tensor.matmul(vg[:, dt, :], xt[:, 1, t, :], wv16[:, 1, :],
                                 start=False, stop=True)
            copy_eng(v16[:, tsl, :], vg[:])

        def qkv_group_thunks(g, copy_eng):
            return [
                lambda: tx_chunk(g, 0, copy_eng),
                lambda: tx_chunk(g, 1, copy_eng),
                lambda: qk_proj(g, wq16, qt, copy_eng),
                lambda: qk_proj(g, wk16, kt16, copy_eng),
                lambda: qkv_v(g, copy_eng),
            ]

        # ---- attention: one 1024-query pair at a time ----
        class PairState:
            pass

        def begin_pair(pr):
            st = PairState()
            st.pr = pr
            st.qa = slice(pr * 2 * QC, pr * 2 * QC + QC)
            st.qb = slice(pr * 2 * QC + QC, (pr + 1) * 2 * QC)
            st.ot = ps_ot.tile([P, 2 * QC], F32, tag="ot")
            st.racc = outp.tile([P, 2 * QC], F16, tag="racc")
            return st

        def kt_scores(st, kt):
            ksl = slice(kt * P, (kt + 1) * P)
            sc = ps_sc.tile([P, 2 * QC], F32, tag="sc")
            nc.tensor.matmul(sc[:, :QC], kt16[:, ksl], qt[:, st.qa],
                             start=True, stop=True)
            nc.tensor.matmul(sc[:, QC:], kt16[:, ksl], qt[:, st.qb],
                             start=True, stop=True)
            e = work.tile([P, 2 * QC], F16, tag="exp")
            nc.scalar.activation(e[:], sc[:], EXP, bias=zbias[:], scale=SCALE)
            return e

        def kt_av(st, kt, e):
            first, last = kt == 0, kt == NT - 1
            nc.tensor.matmul(st.ot[:, :QC], v16[:, kt, :], e[:, :QC],
                             start=first, stop=last)
            nc.tensor.matmul(st.ot[:, QC:], v16[:, kt, :], e[:, QC:],
                             start=first, stop=last)
            if first:
                nc.vector.tensor_copy(st.racc[:], e[:])
            else:
                nc.vector.tensor_add(st.racc[:], st.racc[:], e[:])

        def kloop(st, kts, extra=None):
            kts = list(kts)
            for i, kt in enumerate(kts):
                e = kt_scores(st, kt)
                kt_av(st, kt, e)
                if extra:
                    want = -(-len(extra) // (len(kts) - i))  # ceil-div drain
                    for _ in range(want):
                        extra.pop(0)()

        def finish_pair(st, tail):
            """Emit finish work as a list of thunks.  For the non-tail pair
            these are interleaved into the next pair's k-loop so the PE work
            (row-sum transposes + projections) rides the loop's slack."""
            pr = st.pr
            otb = outp.tile([P, 2 * QC], F16, tag="otb")
            if tail:   # split halves across ScalarE+DVE so proj starts sooner
                nc.scalar.copy(otb[:, :QC], st.ot[:, :QC])
                nc.vector.tensor_copy(otb[:, QC:], st.ot[:, QC:])
            else:
                nc.vector.tensor_copy(otb[:], st.ot[:])
            rt = ps_misc.tile([P, 8], F32, tag="misc")
            recip = outp.tile([P, 8], F32, tag="recip")
            obuf = outp.tile([P, 8, D], F32, tag="obuf")
            thunks = []
            # row sums, directly transposed: rt[q,0] = sum_k racc[k, q]
            for j in range(8):
                thunks.append(lambda j=j: nc.tensor.matmul(
                    rt[:, j:j + 1], st.racc[:, j * P:(j + 1) * P], ones_h[:],
                    start=True, stop=True))
            thunks.append(lambda: nc.vector.reciprocal(recip[:], rt[:]))

            def proj(j):
                t = pr * 8 + j
                pj = ps_misc.tile([P, D], F32, tag="misc")
                nc.tensor.matmul(pj[:], otb[:, j * P:(j + 1) * P], wo16[:],
                                 start=True, stop=True)
                if GPS_STT and tail and j % 2 == 1:
                    # odd tiles: ScalarE stages PSUM->SBUF, GpSimd does the
                    # scale+residual, halving the DVE chain in the tail
                    pjs = outp.tile([P, D], F32, tag="pjs")
                    nc.scalar.copy(pjs[:], pj[:])
                    nc.gpsimd.scalar_tensor_tensor(
                        obuf[:, j, :], pjs[:], recip[:, j:j + 1],
                        xres16[:, t, :], op0=mybir.AluOpType.mult,
                        op1=mybir.AluOpType.add)
                else:
                    nc.vector.scalar_tensor_tensor(
                        obuf[:, j, :], pj[:], recip[:, j:j + 1],
                        xres16[:, t, :], op0=mybir.AluOpType.mult,
                        op1=mybir.AluOpType.add)
                if j % 2 == 1:
                    nc.sync.dma_start(
                        out=out_tiled[:, pr * 8 + j - 1:pr * 8 + j + 1, :],
                        in_=obuf[:, j - 1:j + 1, :])

            for j in range(8):
                thunks.append(lambda j=j: proj(j))
            return thunks

        # ---- schedule ----
        # ScalarE is idle before the exp stream starts: use it for group 0/1
        # copies.  Group 1's V is deferred past kt0 so scoring starts as soon
        # as Q/K of the first two groups exist.
        tx_pair(0)
        tx_pair(1)
        qk_proj(0, wq16, qt, nc.vector.tensor_copy)
        qk_proj(0, wk16, kt16, nc.scalar.copy)
        qk_proj(1, wq16, qt, nc.vector.tensor_copy)
        qk_proj(1, wk16, kt16, nc.scalar.copy)
        st0 = begin_pair(0)
        e0 = kt_scores(st0, 0)
        e1 = kt_scores(st0, 1)
        qkv_v(0, nc.scalar.copy)
        kt_av(st0, 0, e0)
        kt_av(st0, 1, e1)
        qkv_v(1, nc.vector.tensor_copy)
        emit_xres(0)
        kloop(st0, range(2, 7), extra=qkv_group_thunks(2, nc.vector.tensor_copy))
        kloop(st0, range(7, 12), extra=qkv_group_thunks(3, nc.vector.tensor_copy))
        emit_xres(1)
        kloop(st0, range(12, 16))
        fin0 = finish_pair(st0, tail=False)
        st1 = begin_pair(1)
        # pre-issue the first scores so the PE head-of-line wait on pair 0's
        # O^T copy-out doesn't stall the exp stream
        e16 = kt_scores(st1, 0)
        e17 = kt_scores(st1, 1)
        kt_av(st1, 0, e16)
        kt_av(st1, 1, e17)
        kloop(st1, range(2, 16), extra=fin0)
        for th in fin0:
            th()
        fin1 = finish_pair(st1, tail=True)
        for th in fin1:
            th()

    nc.compile()
    return nc


_NC_CACHE = None


def _get_nc():
    global _NC_CACHE
    if _NC_CACHE is None:
        _NC_CACHE = build_bass()
    return _NC_CACHE


def make_in_maps(inputs, W_q, W_k, W_v, W_o, b_o):
    return [
        {
            "inputs": np.ascontiguousarray(inputs[i], dtype=np.float32),
            "W_q": np.asarray(W_q, dtype=np.float32),
            "W_k": np.asarray(W_k, dtype=np.float32),
            "W_v": np.asarray(W_v, dtype=np.float32),
            "W_o": np.asarray(W_o, dtype=np.float32),
            "b_o": np.asarray(b_o, dtype=np.float32),
        }
        for i in range(B)
    ]


def run_sharded(in_maps, trace=False, **kw):
    nc = _get_nc()
    return run_bass_kernel_spmd(nc, in_maps, core_ids=list(range(B)), trace=trace, **kw)


def kernel(inputs, W_q, W_k, W_v, W_o, b_o):
    inputs = np.asarray(inputs)
    res = run_sharded(make_in_maps(inputs, W_q, W_k, W_v, W_o, b_o))
    out = np.stack([np.asarray(res.results[i]["out"]) for i in range(B)], axis=0)
    return out.astype(np.float32)


if __name__ == "__main__":
    rng = np.random.default_rng(0)
    ins = {
        "inputs": rng.standard_normal((B, S, D), dtype=np.float32),
        "W_q": rng.standard_normal((D, U), dtype=np.float32) / 16.0,
        "W_k": rng.standard_normal((D, U), dtype=np.float32) / 16.0,
        "W_v": rng.standard_normal((D, U), dtype=np.float32) / 16.0,
        "W_o": rng.standard_normal((U, D), dtype=np.float32) / np.sqrt(128.0),
        "b_o": np.zeros((D,), dtype=np.float32),
    }
    out = kernel(**ins)
    print("out", out.shape, out.dtype, float(np.abs(out).mean()))


# revision 27
# speedup vs baseline: 1.0363x; 1.0363x over previous
"""Trainium2 Bass kernel for nn_AttentionLayer (B=8, S=2048, D=256, U=128).

Data-parallel over the batch dim: one batch element per NeuronCore, weights
replicated. Per-core flash-attention-style layer in a transpose-free layout.

Sequence relabeling: row s of X lives at (partition p, tile t) with
s = p*NT + t, so DMAs move multi-KB contiguous runs per partition.
Attention is permutation-invariant over sequence position as long as loads,
V/K indexing, residual, and stores use the same relabeling (they do).

Layout/engine plan (v2):
  - X loaded as fp16 via GpSimd casting DMAs (f32->f16 in flight), 4 chunks.
  - X^T built by PE matmuls against an fp16 identity (regular matmul, not
    transpose-mode: streams at ~N cycles and stays in the warm-clock path).
  - Q^T/K^T with W stationary (N=512 moving), V natural with X^T stationary.
  - Scores S^T = K_tile^T^T . Q^T into fp32 PSUM [128,1024] (2 banks),
    exp on ScalarE -> fp16 SBUF (the kernel's pace-setter: 32 x ~1.07us).
  - Row-sum accumulator racc (fp16) += e on DVE: 16-bit packed operands hit
    the 2x_1p DVE mode, one [128,1024] add per key tile.
  - O^T += V_tile^T . E accumulated in fp32 PSUM across key tiles.
  - Finish per 1024-query pair: O^T -> SBUF fp16, row sums via ones-matmul,
    transpose to [q,1] via K=1 matmuls, reciprocal, per-tile projection
    matmul + fused (proj*recip + residual) on DVE, chunked stores.
"""

import sys

if "/opt/trn_rl_repo" not in sys.path:
    sys.path.insert(0, "/opt/trn_rl_repo")

from contextlib import ExitStack

import numpy as np

import concourse.bass as bass
import concourse.tile as tile
from concourse import bacc, mybir
from concourse.bass_utils import run_bass_kernel_spmd
from concourse.masks import make_identity

B, S, D, U, P = 8, 2048, 256, 128, 128
NT = S // P            # 16 sequence tiles of 128
QC = 512               # query chunk (one PSUM bank of fp32)
NPAIR = 2              # two 1024-query pairs
SCALE = 1.0 / float(np.sqrt(U))
F32 = mybir.dt.float32
F16 = mybir.dt.float16
EXP = mybir.ActivationFunctionType.Exp
N_WARMUP = 28          # PE activity while DMAs fly, for HAM warm-up
E_BUFS = 8             # exp output lookahead buffers
GPS_X_DMA = True       # X via gpsimd casting DMA (else SP f32 + DVE cast)
GPS_XRES = True        # residual+bias add on GpSimd (else DVE)
GPS_STT = False        # tail scale+residual split onto GpSimd


def build_bass():
    nc = bacc.Bacc("TRN2", target_bir_lowering=False, debug=False)

    x = nc.dram_tensor("inputs", [S, D], F32, kind="ExternalInput").ap()
    wq_d = nc.dram_tensor("W_q", [D, U], F32, kind="ExternalInput").ap()
    wk_d = nc.dram_tensor("W_k", [D, U], F32, kind="ExternalInput").ap()
    wv_d = nc.dram_tensor("W_v", [D, U], F32, kind="ExternalInput").ap()
    wo_d = nc.dram_tensor("W_o", [U, D], F32, kind="ExternalInput").ap()
    bo_d = nc.dram_tensor("b_o", [D], F32, kind="ExternalInput").ap()
    out_d = nc.dram_tensor("out", [S, D], F32, kind="ExternalOutput").ap()

    # s = p*NT + t: contiguous per-partition runs for every DMA
    x_tiled = x.rearrange("(p t) d -> p t d", t=NT)
    out_tiled = out_d.rearrange("(p t) d -> p t d", t=NT)

    with tile.TileContext(nc) as tc, ExitStack() as ctx:
        consts = ctx.enter_context(tc.tile_pool(name="consts", bufs=1))
        sb = ctx.enter_context(tc.tile_pool(name="sb", bufs=1))
        work = ctx.enter_context(tc.tile_pool(name="work", bufs=E_BUFS))
        outp = ctx.enter_context(tc.tile_pool(name="outp", bufs=2))
        # PSUM budget (8 banks): sc 2x[128,1024]f32 = 4, ot 1x[128,1024]f32
        # = 2, misc 2x[128,512]f32 = 2.
        ps_sc = ctx.enter_context(tc.tile_pool(name="ps_sc", bufs=2, space="PSUM"))
        ps_ot = ctx.enter_context(tc.tile_pool(name="ps_ot", bufs=1, space="PSUM"))
        ps_misc = ctx.enter_context(tc.tile_pool(name="ps_misc", bufs=2, space="PSUM"))

        # ---- tiny constants (DVE memsets, cheap) ----
        zbias = consts.tile([P, 1], F32)
        nc.vector.memset(zbias, 0.0)
        ones_h = consts.tile([P, 1], F16)
        nc.vector.memset(ones_h, 1.0)
        wu_sb = consts.tile([P, P], F16)
        nc.vector.memset(wu_sb, 0.0)

        # ---- loads: everything through GpSimd casting DMAs (f32->f16 in
        # flight, software DGE keeps the HWDGE path free).  Order matters:
        # identity first (gates the transposes), then x chunks 0/1, then
        # Wq/Wk/Wv (needed by group 0's projections), then the rest.
        x16 = sb.tile([P, NT, D], F16)
        # chunk 0 is the critical path: fast HWDGE f32 load + DVE cast beats
        # the software-DGE casting DMA's ~2us extra latency
        x_nat0 = consts.tile([P, 4, D], F32)
        nc.sync.dma_start(out=x_nat0[:], in_=x_tiled[:, 0:4, :])

        ident_h = consts.tile([P, P], F16)
        make_identity(nc, ident_h)

        wq16 = consts.tile([P, 2, U], F16)
        wk16 = consts.tile([P, 2, U], F16)
        wv16 = consts.tile([P, 2, U], F16)
        wo16 = consts.tile([P, D], F16)
        bo16 = consts.tile([P, D], F16)
        bo_bcast = bass.AP(tensor=bo_d.tensor, offset=bo_d.offset,
                           ap=[[0, P]] + list(bo_d.ap))

        # weights are small and early; x chunks 1-3 deliberately queue behind
        # them so chunk 0 (the critical path, on the HWDGE above) gets the
        # DMA queues to itself first
        nc.gpsimd.dma_start(out=wq16[:], in_=wq_d.rearrange("(c p) u -> p c u", p=P))
        nc.gpsimd.dma_start(out=wk16[:], in_=wk_d.rearrange("(c p) u -> p c u", p=P))
        nc.gpsimd.dma_start(out=wv16[:], in_=wv_d.rearrange("(c p) u -> p c u", p=P))
        for g in (1, 2, 3):
            sl = slice(4 * g, 4 * (g + 1))
            nc.gpsimd.dma_start(out=x16[:, sl, :], in_=x_tiled[:, sl, :])
        nc.gpsimd.dma_start(out=wo16[:], in_=wo_d)
        nc.gpsimd.dma_start(out=bo16[:], in_=bo_bcast)
        # split cast across DVE+ScalarE so group 0 unblocks sooner
        nc.vector.tensor_copy(x16[:, 0:2, :], x_nat0[:, 0:2, :])
        nc.scalar.copy(x16[:, 2:4, :], x_nat0[:, 2:4, :])

        # ---- PE warmup while DMAs are in flight ----
        wu_ps = ps_misc.tile([P, P], F32, tag="misc")
        for _ in range(N_WARMUP):
            nc.tensor.matmul(wu_ps[:], wu_sb[:], wu_sb[:], start=True, stop=True)

        # ---- residual (x + b_o) in fp16, off the critical path ----
        xres16 = sb.tile([P, NT, D], F16)
        bo_rep = bass.AP(tensor=bo16.tensor, offset=bo16.offset,
                         ap=[list(bo16.ap[0]), [0, 2]] + list(bo16.ap[1:]))

        def emit_xres(half):
            eng = nc.gpsimd if GPS_XRES else nc.vector
            for i in range(4):
                t0 = half * 8 + 2 * i
                eng.tensor_add(xres16[:, t0:t0 + 2, :],
                               x16[:, t0:t0 + 2, :], bo_rep)

        # ---- per-group: transposes + QKV ----
        xt = sb.tile([P, 2, NT, P], F16)   # X^T chunks: [d_in_chunk, c, t, s]
        qt = sb.tile([P, S], F16)          # Q^T [u, q-col]
        kt16 = sb.tile([P, S], F16)        # K^T [u, k-col]
        v16 = sb.tile([P, NT, U], F16)     # V natural [s_in_tile, t, u]

        def tx_chunk(g, c, copy_eng):
            # 4 transposes via plain matmul against identity into one PSUM
            # bank, then one cast-copy out.
            tsl = slice(4 * g, 4 * (g + 1))
            xtg = ps_misc.tile([P, 4, P], F32, tag="misc")
            for dt in range(4):
                t = 4 * g + dt
                nc.tensor.matmul(xtg[:, dt, :],
                                 x16[:, t, c * P:(c + 1) * P],
                                 ident_h[:], start=True, stop=True)
            copy_eng(xt[:, c, tsl, :], xtg[:])

        def tx_pair(g):
            # head variant: both d-chunks of a group into one (still unused)
            # scores tile, so the misc pool doesn't gate the Q/K matmuls
            tsl = slice(4 * g, 4 * (g + 1))
            xtg = ps_sc.tile([P, 2, 4, P], F32, tag="sc")
            for c in range(2):
                for dt in range(4):
                    t = 4 * g + dt
                    nc.tensor.matmul(xtg[:, c, dt, :],
                                     x16[:, t, c * P:(c + 1) * P],
                                     ident_h[:], start=True, stop=True)
            nc.scalar.copy(xt[:, 0, tsl, :], xtg[:, 0])
            nc.vector.tensor_copy(xt[:, 1, tsl, :], xtg[:, 1])

        def qk_proj(g, w16, dst, copy_eng):
            csl = slice(g * QC, (g + 1) * QC)
            xt2d = xt.rearrange("p c t s -> p c (t s)")
            ps = ps_misc.tile([P, QC], F32, tag="misc")
            nc.tensor.matmul(ps[:], w16[:, 0, :], xt2d[:, 0, csl],
                             start=True, stop=False)
            nc.tensor.matmul(ps[:], w16[:, 1, :], xt2d[:, 1, csl],
                             start=False, stop=True)
            copy_eng(dst[:, csl], ps[:])

        def qkv_qk(g, copy_eng, xt_eng=None):
            tx_chunk(g, 0, copy_eng)
            tx_chunk(g, 1, xt_eng or copy_eng)
            qk_proj(g, wq16, qt, nc.vector.tensor_copy)
            qk_proj(g, wk16, kt16, copy_eng)

        def qkv_v(g, copy_eng):
            tsl = slice(4 * g, 4 * (g + 1))
            vg = ps_misc.tile([P, 4, U], F32, tag="misc")
            for dt in range(4):
                t = 4 * g + dt
                nc.tensor.matmul(vg[:, dt, :], xt[:, 0, t, :], wv16[:, 0, :],
                                 start=True, stop=False)
                nc.tensor.matmul(vg[:, dt, :], xt[:, 1, t, :], wv16[:, 1, :],
                                 start=False, stop=True)
            copy_eng(v16[:, tsl, :], vg[:])

        def qkv_group_thunks(g, copy_eng):
            return [
                lambda: tx_chunk(g, 0, copy_eng),
                lambda: tx_chunk(g, 1, copy_eng),
                lambda: qk_proj(g, wq16, qt, copy_eng),
                lambda: qk_proj(g, wk16, kt16, copy_eng),
                lambda: qkv_v(g, copy_eng),
            ]

        # ---- attention: one 1024-query pair at a time ----
        class PairState:
            pass

        def begin_pair(pr):
            st = PairState()
            st.pr = pr
            st.qa = slice(pr * 2 * QC, pr * 2 * QC + QC)
            st.qb = slice(pr * 2 * QC + QC, (pr + 1) * 2 * QC)
            st.ot = ps_ot.tile([P, 2 * QC], F32, tag="ot")
            st.racc = outp.tile([P, 2 * QC], F16, tag="racc")
            return st

        def kt_scores(st, kt):
            ksl = slice(kt * P, (kt + 1) * P)
            sc = ps_sc.tile([P, 2 * QC], F32, tag="sc")
            nc.tensor.matmul(sc[:, :QC], kt16[:, ksl], qt[:, st.qa],
                             start=True, stop=True)
            nc.tensor.matmul(sc[:, QC:], kt16[:, ksl], qt[:, st.qb],
                             start=True, stop=True)
            e = work.tile([P, 2 * QC], F16, tag="exp")
            nc.scalar.activation(e[:], sc[:], EXP, bias=zbias[:], scale=SCALE)
            return e

        def kt_av(st, kt, e):
            first, last = kt == 0, kt == NT - 1
            nc.tensor.matmul(st.ot[:, :QC], v16[:, kt, :], e[:, :QC],
                             start=first, stop=last)
            nc.tensor.matmul(st.ot[:, QC:], v16[:, kt, :], e[:, QC:],
                             start=first, stop=last)
            if first:
                nc.vector.tensor_copy(st.racc[:], e[:])
            else:
                nc.vector.tensor_add(st.racc[:], st.racc[:], e[:])

        def kloop(st, kts, extra=None):
            kts = list(kts)
            for i, kt in enumerate(kts):
                e = kt_scores(st, kt)
                kt_av(st, kt, e)
                if extra:
                    want = -(-len(extra) // (len(kts) - i))  # ceil-div drain
                    for _ in range(want):
                        extra.pop(0)()

        def finish_pair(st, tail):
            """Emit finish work as a list of thunks.  For the non-tail pair
            these are interleaved into the next pair's k-loop so the PE work
            (row-sum transposes + projections) rides the loop's slack."""
            pr = st.pr
            otb = outp.tile([P, 2 * QC], F16, tag="otb")
            if tail:   # split halves across ScalarE+DVE so proj starts sooner
                nc.scalar.copy(otb[:, :QC], st.ot[:, :QC])
                nc.vector.tensor_copy(otb[:, QC:], st.ot[:, QC:])
            else:
                nc.vector.tensor_copy(otb[:], st.ot[:])
            rt = ps_misc.tile([P, 8], F32, tag="misc")
            recip = outp.tile([P, 8], F32, tag="recip")
            obuf = outp.tile([P, 8, D], F32, tag="obuf")
            thunks = []
            # row sums, directly transposed: rt[q,0] = sum_k racc[k, q]
            for j in range(8):
                thunks.append(lambda j=j: nc.tensor.matmul(
                    rt[:, j:j + 1], st.racc[:, j * P:(j + 1) * P], ones_h[:],
                    start=True, stop=True))
            thunks.append(lambda: nc.vector.reciprocal(recip[:], rt[:]))

            def proj(j):
                t = pr * 8 + j
                pj = ps_misc.tile([P, D], F32, tag="misc")
                nc.tensor.matmul(pj[:], otb[:, j * P:(j + 1) * P], wo16[:],
                                 start=True, stop=True)
                if GPS_STT and tail and j % 2 == 1:
                    # odd tiles: ScalarE stages PSUM->SBUF, GpSimd does the
                    # scale+residual, halving the DVE chain in the tail
                    pjs = outp.tile([P, D], F32, tag="pjs")
                    nc.scalar.copy(pjs[:], pj[:])
                    nc.gpsimd.scalar_tensor_tensor(
                        obuf[:, j, :], pjs[:], recip[:, j:j + 1],
                        xres16[:, t, :], op0=mybir.AluOpType.mult,
                        op1=mybir.AluOpType.add)
                else:
                    nc.vector.scalar_tensor_tensor(
                        obuf[:, j, :], pj[:], recip[:, j:j + 1],
                        xres16[:, t, :], op0=mybir.AluOpType.mult,
                        op1=mybir.AluOpType.add)
                if j % 2 == 1:
                    nc.sync.dma_start(
                        out=out_tiled[:, pr * 8 + j - 1:pr * 8 + j + 1, :],
                        in_=obuf[:, j - 1:j + 1, :])

            for j in range(8):
                thunks.append(lambda j=j: proj(j))
            return thunks

        # ---- schedule ----
        # ScalarE is idle before the exp stream starts: use it for group 0/1
        # copies.  Group 1's V is deferred past kt0 so scoring starts as soon
        # as Q/K of the first two groups exist.
        qkv_qk(0, nc.scalar.copy, xt_eng=nc.vector.tensor_copy)
        qkv_qk(1, nc.scalar.copy, xt_eng=nc.vector.tensor_copy)
        st0 = begin_pair(0)
        e0 = kt_scores(st0, 0)
        e1 = kt_scores(st0, 1)
        qkv_v(0, nc.scalar.copy)
        kt_av(st0, 0, e0)
        kt_av(st0, 1, e1)
        qkv_v(1, nc.vector.tensor_copy)
        emit_xres(0)
        kloop(st0, range(2, 7), extra=qkv_group_thunks(2, nc.vector.tensor_copy))
        kloop(st0, range(7, 12), extra=qkv_group_thunks(3, nc.vector.tensor_copy))
        emit_xres(1)
        kloop(st0, range(12, 16))
        fin0 = finish_pair(st0, tail=False)
        st1 = begin_pair(1)
        # pre-issue the first scores so the PE head-of-line wait on pair 0's
        # O^T copy-out doesn't stall the exp stream
        e16 = kt_scores(st1, 0)
        e17 = kt_scores(st1, 1)
        kt_av(st1, 0, e16)
        kt_av(st1, 1, e17)
        kloop(st1, range(2, 16), extra=fin0)
        for th in fin0:
            th()
        fin1 = finish_pair(st1, tail=True)
        for th in fin1:
            th()

    nc.compile()
    return nc


_NC_CACHE = None


def _get_nc():
    global _NC_CACHE
    if _NC_CACHE is None:
        _NC_CACHE = build_bass()
    return _NC_CACHE


def make_in_maps(inputs, W_q, W_k, W_v, W_o, b_o):
    return [
        {
            "inputs": np.ascontiguousarray(inputs[i], dtype=np.float32),
            "W_q": np.asarray(W_q, dtype=np.float32),
            "W_k": np.asarray(W_k, dtype=np.float32),
            "W_v": np.asarray(W_v, dtype=np.float32),
            "W_o": np.asarray(W_o, dtype=np.float32),
            "b_o": np.asarray(b_o, dtype=np.float32),
        }
        for i in range(B)
    ]


def run_sharded(in_maps, trace=False, **kw):
    nc = _get_nc()
    return run_bass_kernel_spmd(nc, in_maps, core_ids=list(range(B)), trace=trace, **kw)


def kernel(inputs, W_q, W_k, W_v, W_o, b_o):
    inputs = np.asarray(inputs)
    res = run_sharded(make_in_maps(inputs, W_q, W_k, W_v, W_o, b_o))
    out = np.stack([np.asarray(res.results[i]["out"]) for i in range(B)], axis=0)
    return out.astype(np.float32)


if __name__ == "__main__":
    rng = np.random.default_rng(0)
    ins = {
        "inputs": rng.standard_normal((B, S, D), dtype=np.float32),
        "W_q": rng.standard_normal((D, U), dtype=np.float32) / 16.0,
        "W_k": rng.standard_normal((D, U), dtype=np.float32) / 16.0,
        "W_v": rng.standard_normal((D, U), dtype=np.float32) / 16.0,
        "W_o": rng.standard_normal((U, D), dtype=np.float32) / np.sqrt(128.0),
        "b_o": np.zeros((D,), dtype=np.float32),
    }
    out = kernel(**ins)
    print("out", out.shape, out.dtype, float(np.abs(out).mean()))
